# revision 2
# baseline (speedup 1.0000x reference)
"""BoundaryLoss via exp-domain EDT (Gaussian blur on the Tensor engine).

loss = mean(sigmoid(pred) * d),  d = sqrt(EDT2(mask==0)) - sqrt(EDT2(mask!=0))

Key identity: in the exp domain the min-plus EDT recursion becomes an
ordinary separable convolution.  With q = 2^-4,

    E2[i', j''] = sum_{i,j} Z[i, j] * q^{(i-i')^2} * q^{(j-j'')^2}
                = q^{D2[i',j'']} * (1 + eps),   0 <= eps < 1  (one bit)

so D2 is recovered EXACTLY from the fp32 exponent field of E2:
  efield = 127 - 4*D2 + delta, delta in {-1, 0}
  (bits >> 23) - (127.5 - 2^25)  --fp32-rounds-to-->  2^25 - 4*D2
  sqrt activation with scale=-1/4, bias=+2^23 yields sqrt(D2) exactly.

Both blur passes are Tensor-engine matmuls.  Pass 1 uses the DATA as the
stationary operand (lhsT = Z chunk), which fuses the transpose: the output
lands column-major, ready for pass 2 with constant band weights.  The tail
(exponent extract, sqrt, sigmoid, product, accumulate) is split across
Vector and Scalar engines.  Only two activation table sets are loaded
(sigmoid then sqrt), each once.

Sharding: 3 masks per core over 8 cores; host reduces partial sums in f64.
Masks whose max squared distance exceeds 25 (never for random data) and
empty/full masks fall back to an exact host computation.
"""

import numpy as np
import ml_dtypes

import concourse.tile as tile
from concourse import bacc, mybir
from concourse.bass_utils import run_bass_kernel_spmd

H = W = 256
NMASK = 3
NCORES = 8
SQRT_BIAS = -float(2 ** 23)
D2_MAX_OK = 25

_NC_CACHE = {}


def _k1_host():
    """[p=i_local, (t, i')]: K1_t[p, i'] = 2^{-4 (t*128+p-i')^2} (bf16)."""
    p = np.arange(128)
    out = np.zeros((128, 512), np.float64)
    for t in range(2):
        d = (t * 128 + p)[:, None] - np.arange(256)[None, :]
        out[:, t * 256:(t + 1) * 256] = np.exp2(np.maximum(-4.0 * d * d, -400))
    return out.astype(ml_dtypes.bfloat16)


def _k2_host():
    """[p=j_local, (jh, jb, j'')]: 2^{-4 (jh*128+p - jb*128 - j'')^2} (bf16)."""
    p = np.arange(128)
    out = np.zeros((128, 512), np.float64)
    for jh in range(2):
        for jb in range(2):
            d = (jh * 128 + p)[:, None] - (jb * 128 + np.arange(128))[None, :]
            out[:, (jh * 2 + jb) * 128:(jh * 2 + jb + 1) * 128] = np.exp2(
                np.maximum(-4.0 * d * d, -400))
    return out.astype(ml_dtypes.bfloat16)


def build_nc():
    dt = mybir.dt
    f32, bf16, i32 = dt.float32, dt.bfloat16, dt.int32
    AF = mybir.ActivationFunctionType
    OP = mybir.AluOpType

    nc = bacc.Bacc("TRN2", target_bir_lowering=False, debug=False,
                   num_devices=NCORES)
    z_h = nc.dram_tensor("z", [NMASK, 128, 512], bf16, kind="ExternalInput")
    predt_h = nc.dram_tensor("predt", [NMASK, 128, 512], bf16,
                             kind="ExternalInput")
    k1_h = nc.dram_tensor("k1", [128, 512], bf16, kind="ExternalInput")
    k2_h = nc.dram_tensor("k2", [128, 512], bf16, kind="ExternalInput")
    out_h = nc.dram_tensor("out", [128, NMASK + 1], f32,
                           kind="ExternalOutput")

    with tile.TileContext(nc) as tc:
        with (
            tc.tile_pool(name="const", bufs=1) as cp,
            tc.tile_pool(name="work", bufs=2) as wp,
            tc.tile_pool(name="psum", bufs=2, space="PSUM") as psp,
            tc.tile_pool(name="psum1", bufs=1, space="PSUM") as psp1,
        ):
            # input DMAs spread across engine queues so descriptor generation
            # runs in parallel; z and k1 (gating the first matmuls) issue
            # first on otherwise-idle queues
            warm = cp.tile([128, 128], bf16, tag="warm")
            nc.gpsimd.memset(warm[:], 0.0)

            # per-queue DMA bandwidth is only ~50 GB/s: split the tensors
            # that gate the first matmuls (z0, k1) across two queues each
            # and stagger the rest so nothing big serializes behind them
            xs = [cp.tile([128, 1024], bf16, tag=f"X{s}", name=f"X{s}")
                  for s in range(NMASK)]
            k1 = cp.tile([128, 512], bf16, tag="k1")
            k2 = cp.tile([128, 512], bf16, tag="k2")
            pr = cp.tile([128, NMASK * 512], bf16, tag="pr")
            nc.sync.dma_start(xs[0][:, 0:256], z_h.ap()[0][:, 0:256])
            nc.scalar.dma_start(k1[:, 0:256], k1_h.ap()[:, 0:256])
            nc.sync.dma_start(xs[0][:, 256:512], z_h.ap()[0][:, 256:512])
            nc.scalar.dma_start(k1[:, 256:512], k1_h.ap()[:, 256:512])
            nc.gpsimd.dma_start(xs[1][:, 0:512], z_h.ap()[1])
            nc.scalar.dma_start(xs[2][:, 0:512], z_h.ap()[2])
            nc.sync.dma_start(pr[:, 0:512], predt_h.ap()[0])
            nc.scalar.dma_start(k2[:], k2_h.ap())
            nc.sync.dma_start(pr[:, 512:1024], predt_h.ap()[1])
            nc.gpsimd.dma_start(pr[:, 1024:1536], predt_h.ap()[2])

            sqb = cp.tile([128, 1], f32, tag="sqb")
            nc.gpsimd.memset(sqb[:], SQRT_BIAS)

            # PE p-state warmup: dummy matmuls on a zeroed tile during the
            # input-DMA window ramp the Tensor engine to full clock before
            # the real stream starts
            wps = psp1.tile([128, 128], f32, tag="warmps", space="PSUM")
            for _ in range(6):
                nc.tensor.matmul(wps[:], warm[:], warm[:],
                                 start=True, stop=True)

            # pos-polarity builds go first on the otherwise-idle Vector queue
            for s in range(NMASK):
                nc.vector.tensor_scalar(
                    xs[s][:, 512:1024], xs[s][:, 0:512], -1.0, 1.0,
                    op0=OP.mult, op1=OP.add)

            # per-slot sigmoids start as each pred shard lands; the sigmoid
            # table set loads once during the DMA window, the sqrt set loads
            # once before the first slot's sqrt - two loads total
            sg = cp.tile([128, NMASK * 512], f32, tag="sg")
            for s in range(NMASK):
                nc.scalar.activation(sg[:, s * 512:(s + 1) * 512],
                                     pr[:, s * 512:(s + 1) * 512], AF.Sigmoid)

            outsb = cp.tile([128, NMASK + 1], f32, tag="outsb")

            k1v = k1.rearrange("p (t i) -> p t i", t=2)
            k2v = k2.rearrange("p (jh jb j) -> p jh jb j", jh=2, jb=2)

            e1s_tiles = {}

            def emit_pass1(s):
                Xv = xs[s].rearrange("p (pol t j) -> p pol t j", pol=2, t=2)
                e1 = [psp.tile([128, 512], f32, tag=f"e1_{jh}",
                               name=f"e1_{s}_{jh}") for jh in range(2)]
                # t=0 covers the full i' range with start=True; t=1's band
                # only reaches i' >= 120, so its accumulate is trimmed
                for jh in range(2):
                    for pol in range(2):
                        nc.tensor.matmul(
                            e1[jh][:, pol * 256:(pol + 1) * 256],
                            Xv[:, pol, 0, jh * 128:(jh + 1) * 128],
                            k1v[:, 0], start=True, stop=False)
                        nc.tensor.matmul(
                            e1[jh][:, pol * 256 + 120:(pol + 1) * 256],
                            Xv[:, pol, 1, jh * 128:(jh + 1) * 128],
                            k1v[:, 1, 120:256], start=False, stop=True)
                e1s = wp.tile([128, 1024], bf16, tag="e1s", name=f"e1s{s}")
                nc.scalar.copy(e1s[:, 0:512], e1[0][:])
                nc.vector.tensor_copy(e1s[:, 512:1024], e1[1][:])
                e1s_tiles[s] = e1s

            def emit_pass2_and_tail(s, split=False):
                e1s = e1s_tiles.pop(s)
                e2 = psp1.tile([128, 1024], f32, tag="e2", name=f"e2{s}")
                for jb in range(2):
                    for jh in range(2):
                        nc.tensor.matmul(
                            e2[:, jb * 512:(jb + 1) * 512],
                            k2v[:, jh, jb],
                            e1s[:, jh * 512:(jh + 1) * 512],
                            start=(jh == 0), stop=(jh == 1))

                # exponent -> sqrt -> signed distance -> accumulate; the last
                # slot is split per jb half so its drain chain pipelines
                # across Vector and Scalar instead of running serially
                sh = wp.tile([128, 1024], i32, tag="sh", name=f"sh{s}")
                y = wp.tile([128, 1024], f32, tag="y", name=f"y{s}")
                d = wp.tile([128, 1024], f32, tag="d", name=f"d{s}")
                dd = wp.tile([128, 512], f32, tag="dd", name=f"dd{s}")
                dm = wp.tile([128, 512], f32, tag="dm", name=f"dm{s}")
                dv = d.rearrange("p (jb pol i) -> p jb pol i", jb=2, pol=2)
                ddv = dd.rearrange("p (jb i) -> p jb i", jb=2)
                sgv = sg[:, s * 512:(s + 1) * 512].rearrange(
                    "p (jb i) -> p jb i", jb=2)
                dmv = dm.rearrange("p (jb i) -> p jb i", jb=2)
                halves = ((0, 1024),) if not split else ((0, 512), (512, 1024))
                for hi, (lo, hix) in enumerate(halves):
                    nc.vector.tensor_scalar(
                        sh[:, lo:hix], e2[:, lo:hix].bitcast(i32), 23, None,
                        op0=OP.logical_shift_right)
                    # scale is 0.25*(1+2^-10), exact in fp32: biases the
                    # offset strictly into (0, 0.5) for delta in {-1, 0} so
                    # the fp32 add rounds y to exactly 2^23 + D2 (no ties)
                    nc.vector.tensor_scalar(
                        y[:, lo:hix], sh[:, lo:hix], -0.250244140625,
                        float(2 ** 23) + 32.0, op0=OP.mult, op1=OP.add)
                    nc.scalar.activation(d[:, lo:hix], y[:, lo:hix], AF.Sqrt,
                                         scale=1.0, bias=sqb[:])
                    jbs = slice(None) if not split else slice(hi, hi + 1)
                    nc.vector.tensor_tensor(ddv[:, jbs], dv[:, jbs, 1],
                                            dv[:, jbs, 0], op=OP.subtract)
                    nc.vector.tensor_tensor(dmv[:, jbs], ddv[:, jbs],
                                            sgv[:, jbs], op=OP.mult)
                    scr = wp.tile([128, 512], f32, tag="scr",
                                  name=f"scr{s}_{hi}")
                    col = s + hi if split else s
                    nc.scalar.activation(
                        scr[:, 0:(hix - lo) // 2], dm[:, lo // 2:hix // 2],
                        AF.Copy, accum_out=outsb[:, col:col + 1])

            # software-pipelined emission: pass-2 of slot s-1 interleaves
            # after pass-1 of slot s so the PE queue never stalls on the
            # PSUM->SBUF copies
            for s in range(NMASK):
                emit_pass1(s)
                if s >= 1:
                    emit_pass2_and_tail(s - 1)
            emit_pass2_and_tail(NMASK - 1, split=True)

            nc.sync.dma_start(out_h.ap(), outsb[:])
    nc.compile()
    return nc


# ---------------------------------------------------------------------------
# host side

def _row_dist(src):
    n, h, w = src.shape
    big = 10 ** 9
    col = np.arange(w)
    last = np.where(src, col, -big)
    np.maximum.accumulate(last, axis=2, out=last)
    nxt = np.where(src, col, big)
    nxt = np.minimum.accumulate(nxt[:, :, ::-1], axis=2)[:, :, ::-1]
    return np.minimum(np.minimum(col - last, nxt - col), big)


def _exact_d2(src):
    g = _row_dist(src).astype(np.int64)
    g2 = np.minimum(g * g, 10 ** 14)
    d2 = g2.copy()
    cur_max = d2.max()
    for d in range(1, src.shape[1]):
        v = d * d
        if v > cur_max:
            break
        np.minimum(d2[:, d:, :], g2[:, :-d, :] + v, out=d2[:, d:, :])
        np.minimum(d2[:, :-d, :], g2[:, d:, :] + v, out=d2[:, :-d, :])
        cur_max = d2.max()
    return d2


def _host_loss_f64(pred24, z24):
    d2n = _exact_d2(z24)
    d2p = _exact_d2(~z24)
    d = np.sqrt(d2p.astype(np.float64)) - np.sqrt(d2n.astype(np.float64))
    for m in range(z24.shape[0]):
        if not z24[m].any():
            d[m] = 0.0
    sig = 1.0 / (1.0 + np.exp(-pred24.astype(np.float64)))
    return np.float32((sig * d).mean())


def kernel(pred, target):
    pred24 = np.ascontiguousarray(
        np.asarray(pred, dtype=np.float32).reshape(24, H, W))
    targ24 = np.ascontiguousarray(
        np.asarray(target, dtype=np.int32).reshape(24, H, W))
    z24 = targ24 != 0

    if any((not z24[m].any()) or z24[m].all() for m in range(24)):
        return _host_loss_f64(pred24, z24)
    if max(_exact_d2(z24).max(), _exact_d2(~z24).max()) > D2_MAX_OK:
        return _host_loss_f64(pred24, z24)

    if "nc" not in _NC_CACHE:
        _NC_CACHE["nc"] = build_nc()
    nc = _NC_CACHE["nc"]

    k1 = _k1_host()
    k2 = _k2_host()
    in_maps = []
    for c in range(NCORES):
        midx = [s * NCORES + c for s in range(NMASK)]
        zb = (z24[midx]
              .reshape(NMASK, 2, 128, 256).transpose(0, 2, 1, 3)
              .reshape(NMASK, 128, 512).astype(ml_dtypes.bfloat16))
        pt = (pred24[midx].transpose(0, 2, 1)
              .reshape(NMASK, 2, 128, 256).transpose(0, 2, 1, 3)
              .reshape(NMASK, 128, 512).astype(ml_dtypes.bfloat16))
        in_maps.append({
            "z": np.ascontiguousarray(zb),
            "predt": np.ascontiguousarray(pt),
            "k1": k1,
            "k2": k2,
        })
    res = run_bass_kernel_spmd(nc, in_maps, core_ids=list(range(NCORES)))
    total = np.float64(0.0)
    for c in range(NCORES):
        total += np.asarray(res.results[c]["out"], dtype=np.float64).sum()
    return np.float32(total / (24.0 * H * W))


# revision 4
# speedup vs baseline: 1.0973x; 1.0973x over previous
"""BoundaryLoss via exp-domain EDT (Gaussian blur on the Tensor engine).

loss = mean(sigmoid(pred) * d),  d = sqrt(EDT2(mask==0)) - sqrt(EDT2(mask!=0))

Key identity: in the exp domain the min-plus EDT recursion becomes an
ordinary separable convolution.  With q = 2^-4,

    E2[i', j''] = sum_{i,j} Z[i, j] * q^{(i-i')^2} * q^{(j-j'')^2}
                = q^{D2[i',j'']} * (1 + eps),   0 <= eps < 1  (one bit)

so D2 is recovered EXACTLY from the fp32 exponent field of E2:
  efield = 127 - 4*D2 + delta, delta in {-1, 0}
  (bits >> 23) - (127.5 - 2^25)  --fp32-rounds-to-->  2^25 - 4*D2
  sqrt activation with scale=-1/4, bias=+2^23 yields sqrt(D2) exactly.

Both blur passes are Tensor-engine matmuls.  Pass 1 uses the DATA as the
stationary operand (lhsT = Z chunk), which fuses the transpose: the output
lands column-major, ready for pass 2 with constant band weights.  The tail
(exponent extract, sqrt, sigmoid, product, accumulate) is split across
Vector and Scalar engines.  Only two activation table sets are loaded
(sigmoid then sqrt), each once.

Sharding: 3 masks per core over 8 cores; host reduces partial sums in f64.
Masks whose max squared distance exceeds 25 (never for random data) and
empty/full masks fall back to an exact host computation.
"""

import numpy as np
import ml_dtypes

import concourse.tile as tile
from concourse import bacc, mybir
from concourse.bass_utils import run_bass_kernel_spmd
from concourse.tile_rust import add_dep_helper

H = W = 256
NMASK = 3
NCORES = 8
SQRT_BIAS = -float(2 ** 23)
D2_MAX_OK = 25

_NC_CACHE = {}


def _k1_host():
    """[p=i_local, (t, i')]: K1_t[p, i'] = 2^{-4 (t*128+p-i')^2} (bf16)."""
    p = np.arange(128)
    out = np.zeros((128, 512), np.float64)
    for t in range(2):
        d = (t * 128 + p)[:, None] - np.arange(256)[None, :]
        out[:, t * 256:(t + 1) * 256] = np.exp2(np.maximum(-4.0 * d * d, -400))
    return out.astype(ml_dtypes.bfloat16)


def _k2_host():
    """[p=j_local, (jh, jb, j'')]: 2^{-4 (jh*128+p - jb*128 - j'')^2} (bf16)."""
    p = np.arange(128)
    out = np.zeros((128, 512), np.float64)
    for jh in range(2):
        for jb in range(2):
            d = (jh * 128 + p)[:, None] - (jb * 128 + np.arange(128))[None, :]
            out[:, (jh * 2 + jb) * 128:(jh * 2 + jb + 1) * 128] = np.exp2(
                np.maximum(-4.0 * d * d, -400))
    return out.astype(ml_dtypes.bfloat16)


def build_nc():
    dt = mybir.dt
    f32, bf16, i32 = dt.float32, dt.bfloat16, dt.int32
    AF = mybir.ActivationFunctionType
    OP = mybir.AluOpType

    nc = bacc.Bacc("TRN2", target_bir_lowering=False, debug=False,
                   num_devices=NCORES)
    z_h = nc.dram_tensor("z", [NMASK, 128, 512], bf16, kind="ExternalInput")
    predt_h = nc.dram_tensor("predt", [NMASK, 128, 512], bf16,
                             kind="ExternalInput")
    k1_h = nc.dram_tensor("k1", [128, 512], bf16, kind="ExternalInput")
    out_h = nc.dram_tensor("out", [128, NMASK + 1], f32,
                           kind="ExternalOutput")

    with tile.TileContext(nc) as tc:
        with (
            tc.tile_pool(name="const", bufs=1) as cp,
            tc.tile_pool(name="work", bufs=2) as wp,
            tc.tile_pool(name="psum", bufs=2, space="PSUM") as psp,
            tc.tile_pool(name="psum1", bufs=1, space="PSUM") as psp1,
        ):
            # input DMAs spread across engine queues so descriptor generation
            # runs in parallel; z and k1 (gating the first matmuls) issue
            # first on otherwise-idle queues
            warm = cp.tile([128, 128], bf16, tag="warm")
            nc.gpsimd.memset(warm[:], 0.0)

            # per-queue DMA bandwidth is only ~50 GB/s: split the tensors
            # that gate the first matmuls (z0, k1) across two queues each
            # and stagger the rest so nothing big serializes behind them
            xs = [cp.tile([128, 1024], bf16, tag=f"X{s}", name=f"X{s}")
                  for s in range(NMASK)]
            k1 = cp.tile([128, 512], bf16, tag="k1")
            pr = cp.tile([128, NMASK * 512], bf16, tag="pr")
            nc.sync.dma_start(xs[0][:, 0:256], z_h.ap()[0][:, 0:256])
            nc.scalar.dma_start(k1[:, 0:256], k1_h.ap()[:, 0:256])
            nc.sync.dma_start(xs[0][:, 256:512], z_h.ap()[0][:, 256:512])
            nc.scalar.dma_start(k1[:, 256:512], k1_h.ap()[:, 256:512])
            nc.gpsimd.dma_start(xs[1][:, 0:512], z_h.ap()[1])
            nc.scalar.dma_start(xs[2][:, 0:512], z_h.ap()[2])

            sqb = cp.tile([128, 1], f32, tag="sqb")
            nc.gpsimd.memset(sqb[:], SQRT_BIAS)

            # PE p-state warmup: dummy matmuls on a zeroed tile during the
            # input-DMA window ramp the Tensor engine to full clock before
            # the real stream starts
            wps = psp1.tile([128, 128], f32, tag="warmps", space="PSUM")
            for _ in range(12):
                warm_mm = nc.tensor.matmul(wps[:], warm[:], warm[:],
                                           start=True, stop=True)

            # pred DMAs deferred behind the warmups so the z/k1 transfers
            # (which gate the matmul stream) get the full DMA bandwidth
            pred_dmas = [
                nc.sync.dma_start(pr[:, 0:512], predt_h.ap()[0]),
                nc.sync.dma_start(pr[:, 512:1024], predt_h.ap()[1]),
                nc.gpsimd.dma_start(pr[:, 1024:1536], predt_h.ap()[2]),
            ]
            for pdma in pred_dmas:
                add_dep_helper(pdma.ins, warm_mm.ins, sync=True,
                               reason="defer pred load behind z transfers")

            # pos-polarity builds go first on the otherwise-idle Vector queue
            for s in range(NMASK):
                nc.vector.tensor_scalar(
                    xs[s][:, 512:1024], xs[s][:, 0:512], -1.0, 1.0,
                    op0=OP.mult, op1=OP.add)

            # per-slot sigmoids start as each pred shard lands; the sigmoid
            # table set loads once during the DMA window, the sqrt set loads
            # once before the first slot's sqrt - two loads total
            sg = cp.tile([128, NMASK * 512], f32, tag="sg")
            for s in range(NMASK):
                nc.scalar.activation(sg[:, s * 512:(s + 1) * 512],
                                     pr[:, s * 512:(s + 1) * 512], AF.Sigmoid)

            outsb = cp.tile([128, NMASK + 1], f32, tag="outsb")

            k1v = k1.rearrange("p (t i) -> p t i", t=2)
            # pass-2 lhsT blocks are slices of the same Toeplitz band k1:
            # (jh,jb)=(0,0)->k1[t0,0:128], (0,1)->k1[t0,128:256],
            # (1,0)->k1[t1,0:128], (1,1)->k1[t0,0:128]
            k2blk = {
                (0, 0): k1v[:, 0, 0:128],
                (0, 1): k1v[:, 0, 128:256],
                (1, 0): k1v[:, 1, 0:128],
                (1, 1): k1v[:, 0, 0:128],
            }

            e1s_tiles = {}

            def emit_pass1(s):
                Xv = xs[s].rearrange("p (pol t j) -> p pol t j", pol=2, t=2)
                e1 = [psp.tile([128, 512], f32, tag=f"e1_{jh}",
                               name=f"e1_{s}_{jh}") for jh in range(2)]
                # t=0 covers the full i' range with start=True; t=1's band
                # only reaches i' >= 120, so its accumulate is trimmed
                for jh in range(2):
                    for pol in range(2):
                        nc.tensor.matmul(
                            e1[jh][:, pol * 256:(pol + 1) * 256],
                            Xv[:, pol, 0, jh * 128:(jh + 1) * 128],
                            k1v[:, 0], start=True, stop=False)
                        nc.tensor.matmul(
                            e1[jh][:, pol * 256 + 120:(pol + 1) * 256],
                            Xv[:, pol, 1, jh * 128:(jh + 1) * 128],
                            k1v[:, 1, 120:256], start=False, stop=True)
                e1s = wp.tile([128, 1024], bf16, tag="e1s", name=f"e1s{s}")
                nc.scalar.copy(e1s[:, 0:512], e1[0][:])
                nc.vector.tensor_copy(e1s[:, 512:1024], e1[1][:])
                e1s_tiles[s] = e1s

            def emit_pass2_and_tail(s, split=False):
                e1s = e1s_tiles.pop(s)
                e2 = psp1.tile([128, 1024], f32, tag="e2", name=f"e2{s}")
                for jb in range(2):
                    for jh in range(2):
                        nc.tensor.matmul(
                            e2[:, jb * 512:(jb + 1) * 512],
                            k2blk[(jh, jb)],
                            e1s[:, jh * 512:(jh + 1) * 512],
                            start=(jh == 0), stop=(jh == 1))

                # exponent -> sqrt -> signed distance -> accumulate; the last
                # slot is split per jb half so its drain chain pipelines
                # across Vector and Scalar instead of running serially
                sh = wp.tile([128, 1024], i32, tag="sh", name=f"sh{s}")
                y = wp.tile([128, 1024], f32, tag="y", name=f"y{s}")
                d = wp.tile([128, 1024], f32, tag="d", name=f"d{s}")
                dd = wp.tile([128, 512], f32, tag="dd", name=f"dd{s}")
                dm = wp.tile([128, 512], f32, tag="dm", name=f"dm{s}")
                dv = d.rearrange("p (jb pol i) -> p jb pol i", jb=2, pol=2)
                ddv = dd.rearrange("p (jb i) -> p jb i", jb=2)
                sgv = sg[:, s * 512:(s + 1) * 512].rearrange(
                    "p (jb i) -> p jb i", jb=2)
                dmv = dm.rearrange("p (jb i) -> p jb i", jb=2)
                halves = ((0, 1024),) if not split else ((0, 512), (512, 1024))
                for hi, (lo, hix) in enumerate(halves):
                    nc.vector.tensor_scalar(
                        sh[:, lo:hix], e2[:, lo:hix].bitcast(i32), 23, None,
                        op0=OP.logical_shift_right)
                    # scale is 0.25*(1+2^-10), exact in fp32: biases the
                    # offset strictly into (0, 0.5) for delta in {-1, 0} so
                    # the fp32 add rounds y to exactly 2^23 + D2 (no ties)
                    nc.vector.tensor_scalar(
                        y[:, lo:hix], sh[:, lo:hix], -0.250244140625,
                        float(2 ** 23) + 32.0, op0=OP.mult, op1=OP.add)
                    nc.scalar.activation(d[:, lo:hix], y[:, lo:hix], AF.Sqrt,
                                         scale=1.0, bias=sqb[:])
                    jbs = slice(None) if not split else slice(hi, hi + 1)
                    nc.vector.tensor_tensor(ddv[:, jbs], dv[:, jbs, 1],
                                            dv[:, jbs, 0], op=OP.subtract)
                    nc.vector.tensor_tensor(dmv[:, jbs], ddv[:, jbs],
                                            sgv[:, jbs], op=OP.mult)
                    scr = wp.tile([128, 512], f32, tag="scr",
                                  name=f"scr{s}_{hi}")
                    col = s + hi if split else s
                    nc.scalar.activation(
                        scr[:, 0:(hix - lo) // 2], dm[:, lo // 2:hix // 2],
                        AF.Copy, accum_out=outsb[:, col:col + 1])

            # software-pipelined emission: pass-2 of slot s-1 interleaves
            # after pass-1 of slot s so the PE queue never stalls on the
            # PSUM->SBUF copies
            for s in range(NMASK):
                emit_pass1(s)
                if s >= 1:
                    emit_pass2_and_tail(s - 1)
            emit_pass2_and_tail(NMASK - 1, split=True)

            nc.sync.dma_start(out_h.ap(), outsb[:])
    nc.compile()
    return nc


# ---------------------------------------------------------------------------
# host side

def _row_dist(src):
    n, h, w = src.shape
    big = 10 ** 9
    col = np.arange(w)
    last = np.where(src, col, -big)
    np.maximum.accumulate(last, axis=2, out=last)
    nxt = np.where(src, col, big)
    nxt = np.minimum.accumulate(nxt[:, :, ::-1], axis=2)[:, :, ::-1]
    return np.minimum(np.minimum(col - last, nxt - col), big)


def _exact_d2(src):
    g = _row_dist(src).astype(np.int64)
    g2 = np.minimum(g * g, 10 ** 14)
    d2 = g2.copy()
    cur_max = d2.max()
    for d in range(1, src.shape[1]):
        v = d * d
        if v > cur_max:
            break
        np.minimum(d2[:, d:, :], g2[:, :-d, :] + v, out=d2[:, d:, :])
        np.minimum(d2[:, :-d, :], g2[:, d:, :] + v, out=d2[:, :-d, :])
        cur_max = d2.max()
    return d2


def _host_loss_f64(pred24, z24):
    d2n = _exact_d2(z24)
    d2p = _exact_d2(~z24)
    d = np.sqrt(d2p.astype(np.float64)) - np.sqrt(d2n.astype(np.float64))
    for m in range(z24.shape[0]):
        if not z24[m].any():
            d[m] = 0.0
    sig = 1.0 / (1.0 + np.exp(-pred24.astype(np.float64)))
    return np.float32((sig * d).mean())


def kernel(pred, target):
    pred24 = np.ascontiguousarray(
        np.asarray(pred, dtype=np.float32).reshape(24, H, W))
    targ24 = np.ascontiguousarray(
        np.asarray(target, dtype=np.int32).reshape(24, H, W))
    z24 = targ24 != 0

    if any((not z24[m].any()) or z24[m].all() for m in range(24)):
        return _host_loss_f64(pred24, z24)
    if max(_exact_d2(z24).max(), _exact_d2(~z24).max()) > D2_MAX_OK:
        return _host_loss_f64(pred24, z24)

    if "nc" not in _NC_CACHE:
        _NC_CACHE["nc"] = build_nc()
    nc = _NC_CACHE["nc"]

    k1 = _k1_host()
    in_maps = []
    for c in range(NCORES):
        midx = [s * NCORES + c for s in range(NMASK)]
        zb = (z24[midx]
              .reshape(NMASK, 2, 128, 256).transpose(0, 2, 1, 3)
              .reshape(NMASK, 128, 512).astype(ml_dtypes.bfloat16))
        pt = (pred24[midx].transpose(0, 2, 1)
              .reshape(NMASK, 2, 128, 256).transpose(0, 2, 1, 3)
              .reshape(NMASK, 128, 512).astype(ml_dtypes.bfloat16))
        in_maps.append({
            "z": np.ascontiguousarray(zb),
            "predt": np.ascontiguousarray(pt),
            "k1": k1,
        })
    res = run_bass_kernel_spmd(nc, in_maps, core_ids=list(range(NCORES)))
    total = np.float64(0.0)
    for c in range(NCORES):
        total += np.asarray(res.results[c]["out"], dtype=np.float64).sum()
    return np.float32(total / (24.0 * H * W))
